# revision 18
# baseline (speedup 1.0000x reference)
"""Trainium2 Bass kernel for CoRA/AdaLoRA embedding lookup.

Computes: out = (E + scaling * lora_B @ (lora_A * mask))[x]  for
  E [500000, 128] f32, lora_B [500000, 8] f32, lora_A [8, 128] f32,
  rank_pattern [8] f32, x [4096, 200] int.

Strategy: pure data-parallel over the batch across 8 NeuronCores with the
table replicated.  Per core, tokens are bucketed by vocab bank (16 banks of
31250 rows, so in-bank indices fit int16) on the host.  Rows of a fused
table [E | lora_B | pad] (768 B, dma_gather needs elem%256B==0) are gathered
with gpsimd.dma_gather in chunks of 1024 indices (ucode descriptor-ring
limit) with -1 tail padding (skipped by HW).  The rank-8 LoRA delta is
computed on-chip (PE transpose + block-diagonal K=64 matmul) and added in
place to the gathered embedding columns, which are DMAd out per bank.  The
host un-permutes the sorted output.  Per-core HBM traffic ~140 MB.
"""

import numpy as np

V = 500000
D = 128
R = 8
EROW = 192             # fused row: 128 emb + 8 lora_B + 56 pad = 768 B
SCALING = 2.0          # LORA_ALPHA / R = 16 / 8
THRESH = 0.1
B, L = 4096, 200
NCORES = 8
P = 128
TPC = B * L // NCORES  # 102400 tokens per core

NBANK = 16
BW = V // NBANK        # 31250 (< 2^15, in-bank index fits int16)
NSUB = 7               # chunks (= compute subtiles) per bank
G = 8                  # dst columns (of 128 slots) per chunk
CHUNK = G * P          # 1024 idxs per dma_gather (HW ring limit)
CAP = NSUB * CHUNK     # 7168 slots per bank (static capacity)
CCOL = CAP // P        # 56 dst columns per bank
ICOL = CAP // 16       # 448 idx columns per bank
NCOL = NBANK * CCOL    # 896 total out columns


def build_nc(nbank=NBANK, bw=BW, nsub=NSUB):
    from concourse import bass, bacc, mybir
    from concourse.library_config import mlp
    from contextlib import ExitStack

    f32 = mybir.dt.float32
    i16 = mybir.dt.int16
    cap = nsub * CHUNK
    ccol = cap // P
    icol = cap // 16
    ichk = CHUNK // 16  # 64 idx columns per chunk
    v = nbank * bw
    nsubt = nbank * nsub  # total subtiles == total gathers

    nc = bacc.Bacc()
    tab = nc.declare_dram_parameter("tab", [v, EROW], f32, False)
    idx = nc.declare_dram_parameter("idx", [P, nbank * icol], i16, False)
    cnts = nc.declare_dram_parameter("cnts", [1, nbank * nsub], mybir.dt.int32, False)
    aeffb = nc.declare_dram_parameter("aeffb", [G * R, G * D], f32, False)
    ident = nc.declare_dram_parameter("ident", [P, P], f32, False)
    out = nc.declare_dram_parameter("out", [P, nbank * ccol, D], f32, True)

    with ExitStack() as st:
        block = st.enter_context(nc.Block())
        idx_sb = st.enter_context(nc.sbuf_tensor("idx_sb", [P, nbank * icol], i16))
        aug = [
            st.enter_context(nc.sbuf_tensor(f"aug{i}", [P, ccol, EROW], f32))
            for i in range(2)
        ]
        lb_cont = [
            st.enter_context(nc.sbuf_tensor(f"lbc{i}", [P, G * R], f32))
            for i in range(2)
        ]
        lbT = [
            st.enter_context(nc.sbuf_tensor(f"lbT{i}", [G * R, P], f32))
            for i in range(2)
        ]
        ident_sb = st.enter_context(nc.sbuf_tensor("ident_sb", [P, P], f32))
        aeff_sb = st.enter_context(nc.sbuf_tensor("aeff_sb", [G * R, G * D], f32))
        pt_full = [
            st.enter_context(nc.psum_tensor(f"pt{i}", [G * R, 512], f32))
            for i in range(2)
        ]
        pm = [
            [
                st.enter_context(nc.psum_tensor(f"pm{i}_{q}", [P, 512], f32))
                for q in range(2)
            ]
            for i in range(2)
        ]
        io_sem = st.enter_context(nc.semaphore("io_sem"))
        ix_sem = st.enter_context(nc.semaphore("ix_sem"))
        z_sem = st.enter_context(nc.semaphore("z_sem"))
        g_sems = [st.enter_context(nc.semaphore(f"g_sem{i}")) for i in range(2)]
        o_sem = st.enter_context(nc.semaphore("o_sem"))
        d1_sem = st.enter_context(nc.semaphore("d1_sem"))
        d2_sem = st.enter_context(nc.semaphore("d2_sem"))
        d3_sem = st.enter_context(nc.semaphore("d3_sem"))
        pe_sem = st.enter_context(nc.semaphore("pe_sem"))

        @block.gpsimd
        def _(gp: "bass.BassGpSimd"):
            gp.load_library(mlp)
            gp.wait_ge(ix_sem, 16)  # idx loaded
            gp.wait_ge(z_sem, 2)    # aug tiles zeroed
            with gp.register("cnt") as cnt_reg:
                for b in range(nbank):
                    pe_ = b % 2
                    if b >= 2:
                        gp.wait_ge(o_sem, 16 * (b - 1))  # out DMA of bank b-2 done
                    for s in range(nsub):
                        k = b * nsub + s
                        gp.reg_load(cnt_reg, cnts[0:1, k : k + 1])
                        cnt = gp.snap(cnt_reg)
                        gp.dma_gather(
                            aug[pe_][:, s * G : (s + 1) * G, :],
                            tab[b * bw : (b + 1) * bw, :],
                            idx_sb[:, b * icol + s * ichk : b * icol + (s + 1) * ichk],
                            CHUNK,
                            cnt,
                            EROW,
                        ).then_inc(g_sems[pe_], 16)

        @block.vector
        def _(ve: "bass.BassVectorEngine"):
            for i in range(2):
                ve.memset(aug[i][:, :, :], 0.0).then_inc(z_sem, 1)
            # prologue: lb_cont for subtile 0 (whole bank 0 gathered)
            ve.wait_ge(g_sems[0], 16 * nsub)
            ve.tensor_copy(
                out=lb_cont[0][:, :], in_=aug[0][:, 0:G, D : D + R]
            ).then_inc(d1_sem, 1)
            for n in range(nsubt):
                b, s = divmod(n, nsub)
                pe_ = b % 2
                # lbT copy (needs PE transpose n)
                ve.wait_ge(pe_sem, 2 * n + 1)
                ve.tensor_copy(
                    out=lbT[n % 2][:, :], in_=pt_full[n % 2][:, 0:P]
                ).then_inc(d2_sem, 1)
                # software-pipelined lb_cont for subtile n+1
                if n + 1 < nsubt:
                    b2, s2 = divmod(n + 1, nsub)
                    if s2 == 0:
                        ve.wait_ge(g_sems[b2 % 2], 16 * nsub * (b2 // 2 + 1))
                    ve.tensor_copy(
                        out=lb_cont[(n + 1) % 2][:, :],
                        in_=aug[b2 % 2][:, s2 * G : (s2 + 1) * G, D : D + R],
                    ).then_inc(d1_sem, 1)
                # adds (need PE matmuls n); in-place into the emb columns
                ve.wait_ge(pe_sem, 2 * n + 2)
                half = G // 2
                ve.tensor_add(
                    out=aug[pe_][:, s * G : s * G + half, 0:D],
                    in0=aug[pe_][:, s * G : s * G + half, 0:D],
                    in1=pm[n % 2][0][:, :],
                )
                ve.tensor_add(
                    out=aug[pe_][:, s * G + half : (s + 1) * G, 0:D],
                    in0=aug[pe_][:, s * G + half : (s + 1) * G, 0:D],
                    in1=pm[n % 2][1][:, :],
                ).then_inc(d3_sem, 1)

        @block.tensor
        def _(te: "bass.BassTensorEngine"):
            te.wait_ge(io_sem, 32)  # ident + aeff loaded
            for n in range(nsubt):
                te.wait_ge(d1_sem, n + 1)
                if n >= 2:
                    te.wait_ge(d2_sem, n - 1)  # WAR pt[n%2]
                te.transpose(
                    out=pt_full[n % 2][:, 0:P],
                    in_=lb_cont[n % 2][:, :],
                    identity=ident_sb[:, :],
                ).then_inc(pe_sem, 1)
                te.wait_ge(d2_sem, n + 1)      # lbT ready
                if n >= 2:
                    te.wait_ge(d3_sem, n - 1)  # WAR pm[n%2]
                te.matmul(
                    out=pm[n % 2][0][:, :],
                    lhsT=lbT[n % 2][:, :],
                    rhs=aeff_sb[:, 0:512],
                    start=True,
                    stop=True,
                )
                te.matmul(
                    out=pm[n % 2][1][:, :],
                    lhsT=lbT[n % 2][:, :],
                    rhs=aeff_sb[:, 512:1024],
                    start=True,
                    stop=True,
                ).then_inc(pe_sem, 1)

        @block.sync
        def _(sy: "bass.BassEngine"):
            sy.dma_start(out=idx_sb[:, :], in_=idx[:, :]).then_inc(ix_sem, 16)
            sy.dma_start(out=ident_sb[:, :], in_=ident[:, :]).then_inc(io_sem, 16)
            sy.dma_start(out=aeff_sb[:, :], in_=aeffb[:, :]).then_inc(io_sem, 16)
            for b in range(nbank):
                sy.wait_ge(d3_sem, nsub * (b + 1))
                sy.dma_start(
                    out=out[:, b * ccol : (b + 1) * ccol, :],
                    in_=aug[b % 2][:, :, 0:D],
                ).then_inc(o_sem, 16)
            sy.wait_ge(o_sem, 16 * nbank)

    nc.compile()
    return nc


_NC_CACHE = {}


def _get_nc():
    if "nc" not in _NC_CACHE:
        _NC_CACHE["nc"] = build_nc()
    return _NC_CACHE["nc"]


def _wrap16(lst):
    """Token t -> (t % 16, t // 16), tiled 8x across 128 partitions."""
    blk = lst.reshape(-1, 16).T  # [16, n/16]
    return np.tile(blk, (8, 1))


def prepare_in_maps(x, embedding_weight, lora_A, lora_B, rank_pattern):
    x = np.asarray(x)
    E = np.asarray(embedding_weight, dtype=np.float32)
    A = np.asarray(lora_A, dtype=np.float32)
    LB = np.asarray(lora_B, dtype=np.float32)
    rp = np.asarray(rank_pattern, dtype=np.float32)

    a_scaled = A * (rp > THRESH).astype(np.float32)[:, None] * np.float32(SCALING)
    aeffb = np.zeros((G * R, G * D), dtype=np.float32)
    for gg in range(G):
        aeffb[gg * R : (gg + 1) * R, gg * D : (gg + 1) * D] = a_scaled
    tab = np.zeros((V, EROW), dtype=np.float32)
    tab[:, :D] = E
    tab[:, D : D + R] = LB
    ident = np.eye(P, dtype=np.float32)

    xi = x.reshape(-1).astype(np.int64)
    in_maps = []
    host_info = []
    for c in range(NCORES):
        xc = xi[c * TPC : (c + 1) * TPC]
        bank = xc // BW
        within = (xc - bank * BW).astype(np.int16)
        order = np.argsort(bank, kind="stable")
        counts = np.bincount(bank, minlength=NBANK).astype(np.int64)
        overflow = {}
        idx16 = np.full((P, NBANK * ICOL), -1, dtype=np.int16)
        takes = np.zeros(NBANK * NSUB, dtype=np.int32)
        start = 0
        for b in range(NBANK):
            nb = int(counts[b])
            take = min(nb, CAP)
            lst = np.full(CAP, -1, dtype=np.int16)
            lst[:take] = within[order[start : start + take]]
            if nb > CAP:  # pathological: handle the excess on the host
                overflow[b] = order[start + take : start + nb]
            # per-chunk valid counts (chunks are filled front to back)
            for s in range(NSUB):
                t = min(max(take - s * CHUNK, 0), CHUNK)
                if t == 0:  # ucode needs >=1 valid index; slot is discarded
                    lst[s * CHUNK] = 0
                    t = 1
                takes[b * NSUB + s] = t
            idx16[:, b * ICOL : (b + 1) * ICOL] = _wrap16(lst)
            start += nb
        in_maps.append(
            {
                "tab": tab,
                "idx": idx16,
                "cnts": takes.reshape(1, NBANK * NSUB),
                "aeffb": aeffb,
                "ident": ident,
            }
        )
        host_info.append((order, counts, overflow))
    return in_maps, host_info, (E, LB, a_scaled)


def collect(results, host_info, tabs, x):
    """Un-sort the banked output; host-patches (never-in-practice) bank overflow."""
    E, LB, a_scaled = tabs
    xi = np.asarray(x).reshape(-1).astype(np.int64)
    cores = []
    for c in range(NCORES):
        order, counts, overflow = host_info[c]
        oc = np.asarray(results[c]["out"])
        flat = oc.transpose(1, 0, 2).reshape(NCOL * P, D)
        core_out = np.empty((TPC, D), dtype=np.float32)
        src_slots = np.concatenate(
            [np.arange(min(int(counts[b]), CAP)) + b * CAP for b in range(NBANK)]
        )
        starts = np.concatenate([[0], np.cumsum(counts)]).astype(np.int64)
        dst_tok = np.concatenate(
            [order[starts[b] : starts[b] + min(int(counts[b]), CAP)] for b in range(NBANK)]
        )
        core_out[dst_tok] = flat[src_slots]
        for b, toks in overflow.items():
            ids = xi[c * TPC + toks]
            core_out[toks] = E[ids] + LB[ids] @ a_scaled
        cores.append(core_out)
    return np.concatenate(cores, axis=0).reshape(B, L, D)


def kernel(x, embedding_weight, lora_A, lora_B, rank_pattern):
    from concourse.bass_utils import run_bass_kernel_spmd

    x = np.asarray(x)
    in_maps, host_info, tabs = prepare_in_maps(
        x, embedding_weight, lora_A, lora_B, rank_pattern
    )
    nc = _get_nc()
    res = run_bass_kernel_spmd(nc, in_maps, list(range(NCORES))).results
    return collect(res, host_info, tabs, x)


# revision 20
# speedup vs baseline: 1.2057x; 1.2057x over previous
"""Trainium2 Bass kernel for CoRA/AdaLoRA embedding lookup.

Computes: out = (E + scaling * lora_B @ (lora_A * mask))[x]  for
  E [500000, 128] f32, lora_B [500000, 8] f32, lora_A [8, 128] f32,
  rank_pattern [8] f32, x [4096, 200] int.

Strategy: pure data-parallel over the batch across 8 NeuronCores with the
table replicated.  Per core, tokens are bucketed by vocab bank (16 banks of
31250 rows, so in-bank indices fit int16) on the host.  Rows of a fused
table [E | lora_B | pad] (768 B, dma_gather needs elem%256B==0) are gathered
with gpsimd.dma_gather in chunks of 1024 indices (ucode descriptor-ring
limit) with -1 tail padding (skipped by HW).  The rank-8 LoRA delta is
computed on-chip (PE transpose + block-diagonal K=64 matmul) and added in
place to the gathered embedding columns, which are DMAd out per bank.  The
host un-permutes the sorted output.  Per-core HBM traffic ~140 MB.
"""

import numpy as np

V = 500000
D = 128
R = 8
EROW = 192             # fused row: 128 emb + 8 lora_B + 56 pad = 768 B
SCALING = 2.0          # LORA_ALPHA / R = 16 / 8
THRESH = 0.1
B, L = 4096, 200
NCORES = 8
P = 128
TPC = B * L // NCORES  # 102400 tokens per core

NBANK = 16
BW = V // NBANK        # 31250 (< 2^15, in-bank index fits int16)
NSUB = 7               # chunks (= compute subtiles) per bank
G = 8                  # dst columns (of 128 slots) per chunk
CHUNK = G * P          # 1024 idxs per dma_gather (HW ring limit)
CAP = NSUB * CHUNK     # 7168 slots per bank (static capacity)
CCOL = CAP // P        # 56 dst columns per bank
ICOL = CAP // 16       # 448 idx columns per bank
NCOL = NBANK * CCOL    # 896 total out columns


def build_nc(nbank=NBANK, bw=BW, nsub=NSUB):
    from concourse import bass, bacc, mybir
    from concourse.library_config import mlp
    from contextlib import ExitStack

    f32 = mybir.dt.float32
    bf16 = mybir.dt.bfloat16
    i16 = mybir.dt.int16
    cap = nsub * CHUNK
    ccol = cap // P
    icol = cap // 16
    ichk = CHUNK // 16  # 64 idx columns per chunk
    v = nbank * bw
    nsubt = nbank * nsub  # total subtiles == total gathers

    nc = bacc.Bacc(num_swdge_queues=2)
    tab = nc.declare_dram_parameter("tab", [v, EROW], f32, False)
    idx = nc.declare_dram_parameter("idx", [P, nbank * icol], i16, False)
    cnts = nc.declare_dram_parameter("cnts", [1, nbank * nsub], mybir.dt.int32, False)
    aeffb = nc.declare_dram_parameter("aeffb", [G * R, G * D], bf16, False)
    ident = nc.declare_dram_parameter("ident", [P, P], f32, False)
    out = nc.declare_dram_parameter("out", [P, nbank * ccol, D], f32, True)

    with ExitStack() as st:
        block = st.enter_context(nc.Block())
        idx_sb = st.enter_context(nc.sbuf_tensor("idx_sb", [P, nbank * icol], i16))
        cnts_sb = st.enter_context(
            nc.sbuf_tensor("cnts_sb", [1, nbank * nsub], mybir.dt.int32)
        )
        aug = [
            st.enter_context(nc.sbuf_tensor(f"aug{i}", [P, ccol, EROW], f32))
            for i in range(2)
        ]
        lb_cont = [
            st.enter_context(nc.sbuf_tensor(f"lbc{i}", [P, G * R], f32))
            for i in range(2)
        ]
        lbT = [
            st.enter_context(nc.sbuf_tensor(f"lbT{i}", [G * R, P], bf16))
            for i in range(2)
        ]
        ident_sb = st.enter_context(nc.sbuf_tensor("ident_sb", [P, P], f32))
        aeff_sb = st.enter_context(nc.sbuf_tensor("aeff_sb", [G * R, G * D], bf16))
        pt_full = [
            st.enter_context(nc.psum_tensor(f"pt{i}", [G * R, 512], f32))
            for i in range(2)
        ]
        pm = [
            [
                st.enter_context(nc.psum_tensor(f"pm{i}_{q}", [P, 512], f32))
                for q in range(2)
            ]
            for i in range(2)
        ]
        io_sem = st.enter_context(nc.semaphore("io_sem"))
        ix_sem = st.enter_context(nc.semaphore("ix_sem"))
        z_sem = st.enter_context(nc.semaphore("z_sem"))
        g_sems = [st.enter_context(nc.semaphore(f"g_sem{i}")) for i in range(2)]
        o_sem = st.enter_context(nc.semaphore("o_sem"))
        d1_sem = st.enter_context(nc.semaphore("d1_sem"))
        d2_sem = st.enter_context(nc.semaphore("d2_sem"))
        d3_sem = st.enter_context(nc.semaphore("d3_sem"))
        pe_sem = st.enter_context(nc.semaphore("pe_sem"))

        @block.gpsimd
        def _(gp: "bass.BassGpSimd"):
            gp.load_library(mlp)
            gp.wait_ge(ix_sem, 32)  # idx + counts loaded
            gp.wait_ge(z_sem, 2)    # aug tiles zeroed
            with gp.register("cnt") as cnt_reg:
                for b in range(nbank):
                    pe_ = b % 2
                    if b >= 2:
                        gp.wait_ge(o_sem, 16 * (b - 1))  # out DMA of bank b-2 done
                    for s in range(nsub):
                        k = b * nsub + s
                        gp.reg_load(cnt_reg, cnts_sb[0:1, k : k + 1])
                        cnt = gp.snap(cnt_reg)
                        gp.dma_gather(
                            aug[pe_][:, s * G : (s + 1) * G, :],
                            tab[b * bw : (b + 1) * bw, :],
                            idx_sb[:, b * icol + s * ichk : b * icol + (s + 1) * ichk],
                            CHUNK,
                            cnt,
                            EROW,
                            queue_num=pe_,
                        ).then_inc(g_sems[pe_], 16)

        @block.vector
        def _(ve: "bass.BassVectorEngine"):
            for i in range(2):
                ve.memset(aug[i][:, :, :], 0.0).then_inc(z_sem, 1)
            # prologue: lb_cont for subtile 0 (whole bank 0 gathered)
            ve.wait_ge(g_sems[0], 16 * nsub)
            ve.tensor_copy(
                out=lb_cont[0][:, :], in_=aug[0][:, 0:G, D : D + R]
            ).then_inc(d1_sem, 1)
            for n in range(nsubt):
                b, s = divmod(n, nsub)
                pe_ = b % 2
                # lbT copy (needs PE transpose n)
                ve.wait_ge(pe_sem, 2 * n + 1)
                ve.tensor_copy(
                    out=lbT[n % 2][:, :], in_=pt_full[n % 2][:, 0:P]
                ).then_inc(d2_sem, 1)
                # software-pipelined lb_cont for subtile n+1
                if n + 1 < nsubt:
                    b2, s2 = divmod(n + 1, nsub)
                    if s2 == 0:
                        ve.wait_ge(g_sems[b2 % 2], 16 * nsub * (b2 // 2 + 1))
                    ve.tensor_copy(
                        out=lb_cont[(n + 1) % 2][:, :],
                        in_=aug[b2 % 2][:, s2 * G : (s2 + 1) * G, D : D + R],
                    ).then_inc(d1_sem, 1)
                # adds (need PE matmuls n); in-place into the emb columns
                ve.wait_ge(pe_sem, 2 * n + 2)
                half = G // 2
                ve.tensor_add(
                    out=aug[pe_][:, s * G : s * G + half, 0:D],
                    in0=aug[pe_][:, s * G : s * G + half, 0:D],
                    in1=pm[n % 2][0][:, :],
                )
                ve.tensor_add(
                    out=aug[pe_][:, s * G + half : (s + 1) * G, 0:D],
                    in0=aug[pe_][:, s * G + half : (s + 1) * G, 0:D],
                    in1=pm[n % 2][1][:, :],
                ).then_inc(d3_sem, 1)

        @block.tensor
        def _(te: "bass.BassTensorEngine"):
            te.wait_ge(io_sem, 32)  # ident + aeff loaded
            for n in range(nsubt):
                te.wait_ge(d1_sem, n + 1)
                if n >= 2:
                    te.wait_ge(d2_sem, n - 1)  # WAR pt[n%2]
                te.transpose(
                    out=pt_full[n % 2][:, 0:P],
                    in_=lb_cont[n % 2][:, :],
                    identity=ident_sb[:, :],
                ).then_inc(pe_sem, 1)
                te.wait_ge(d2_sem, n + 1)      # lbT ready
                if n >= 2:
                    te.wait_ge(d3_sem, n - 1)  # WAR pm[n%2]
                te.matmul(
                    out=pm[n % 2][0][:, :],
                    lhsT=lbT[n % 2][:, :],
                    rhs=aeff_sb[:, 0:512],
                    start=True,
                    stop=True,
                )
                te.matmul(
                    out=pm[n % 2][1][:, :],
                    lhsT=lbT[n % 2][:, :],
                    rhs=aeff_sb[:, 512:1024],
                    start=True,
                    stop=True,
                ).then_inc(pe_sem, 1)

        @block.sync
        def _(sy: "bass.BassEngine"):
            sy.dma_start(out=idx_sb[:, :], in_=idx[:, :]).then_inc(ix_sem, 16)
            sy.dma_start(out=cnts_sb[:, :], in_=cnts[:, :]).then_inc(ix_sem, 16)
            sy.dma_start(out=ident_sb[:, :], in_=ident[:, :]).then_inc(io_sem, 16)
            sy.dma_start(out=aeff_sb[:, :], in_=aeffb[:, :]).then_inc(io_sem, 16)
            for b in range(nbank):
                sy.wait_ge(d3_sem, nsub * (b + 1))
                sy.dma_start(
                    out=out[:, b * ccol : (b + 1) * ccol, :],
                    in_=aug[b % 2][:, :, 0:D],
                ).then_inc(o_sem, 16)
            sy.wait_ge(o_sem, 16 * nbank)

    nc.compile()
    return nc


_NC_CACHE = {}


def _get_nc():
    if "nc" not in _NC_CACHE:
        _NC_CACHE["nc"] = build_nc()
    return _NC_CACHE["nc"]


def _wrap16(lst):
    """Token t -> (t % 16, t // 16), tiled 8x across 128 partitions."""
    blk = lst.reshape(-1, 16).T  # [16, n/16]
    return np.tile(blk, (8, 1))


def prepare_in_maps(x, embedding_weight, lora_A, lora_B, rank_pattern):
    x = np.asarray(x)
    E = np.asarray(embedding_weight, dtype=np.float32)
    A = np.asarray(lora_A, dtype=np.float32)
    LB = np.asarray(lora_B, dtype=np.float32)
    rp = np.asarray(rank_pattern, dtype=np.float32)

    import ml_dtypes

    a_scaled = A * (rp > THRESH).astype(np.float32)[:, None] * np.float32(SCALING)
    aeffb = np.zeros((G * R, G * D), dtype=ml_dtypes.bfloat16)
    for gg in range(G):
        aeffb[gg * R : (gg + 1) * R, gg * D : (gg + 1) * D] = a_scaled
    tab = np.zeros((V, EROW), dtype=np.float32)
    tab[:, :D] = E
    tab[:, D : D + R] = LB
    ident = np.eye(P, dtype=np.float32)

    xi = x.reshape(-1).astype(np.int64)
    in_maps = []
    host_info = []
    for c in range(NCORES):
        xc = xi[c * TPC : (c + 1) * TPC]
        bank = xc // BW
        within = (xc - bank * BW).astype(np.int16)
        order = np.argsort(bank, kind="stable")
        counts = np.bincount(bank, minlength=NBANK).astype(np.int64)
        overflow = {}
        idx16 = np.full((P, NBANK * ICOL), -1, dtype=np.int16)
        takes = np.zeros(NBANK * NSUB, dtype=np.int32)
        start = 0
        for b in range(NBANK):
            nb = int(counts[b])
            take = min(nb, CAP)
            lst = np.full(CAP, -1, dtype=np.int16)
            lst[:take] = within[order[start : start + take]]
            if nb > CAP:  # pathological: handle the excess on the host
                overflow[b] = order[start + take : start + nb]
            # per-chunk valid counts (chunks are filled front to back)
            for s in range(NSUB):
                t = min(max(take - s * CHUNK, 0), CHUNK)
                if t == 0:  # ucode needs >=1 valid index; slot is discarded
                    lst[s * CHUNK] = 0
                    t = 1
                takes[b * NSUB + s] = t
            idx16[:, b * ICOL : (b + 1) * ICOL] = _wrap16(lst)
            start += nb
        in_maps.append(
            {
                "tab": tab,
                "idx": idx16,
                "cnts": takes.reshape(1, NBANK * NSUB),
                "aeffb": aeffb,
                "ident": ident,
            }
        )
        host_info.append((order, counts, overflow))
    return in_maps, host_info, (E, LB, a_scaled)


def collect(results, host_info, tabs, x):
    """Un-sort the banked output; host-patches (never-in-practice) bank overflow."""
    E, LB, a_scaled = tabs
    xi = np.asarray(x).reshape(-1).astype(np.int64)
    cores = []
    for c in range(NCORES):
        order, counts, overflow = host_info[c]
        oc = np.asarray(results[c]["out"])
        flat = oc.transpose(1, 0, 2).reshape(NCOL * P, D)
        core_out = np.empty((TPC, D), dtype=np.float32)
        src_slots = np.concatenate(
            [np.arange(min(int(counts[b]), CAP)) + b * CAP for b in range(NBANK)]
        )
        starts = np.concatenate([[0], np.cumsum(counts)]).astype(np.int64)
        dst_tok = np.concatenate(
            [order[starts[b] : starts[b] + min(int(counts[b]), CAP)] for b in range(NBANK)]
        )
        core_out[dst_tok] = flat[src_slots]
        for b, toks in overflow.items():
            ids = xi[c * TPC + toks]
            core_out[toks] = E[ids] + LB[ids] @ a_scaled
        cores.append(core_out)
    return np.concatenate(cores, axis=0).reshape(B, L, D)


def kernel(x, embedding_weight, lora_A, lora_B, rank_pattern):
    from concourse.bass_utils import run_bass_kernel_spmd

    x = np.asarray(x)
    in_maps, host_info, tabs = prepare_in_maps(
        x, embedding_weight, lora_A, lora_B, rank_pattern
    )
    nc = _get_nc()
    res = run_bass_kernel_spmd(nc, in_maps, list(range(NCORES))).results
    return collect(res, host_info, tabs, x)


# revision 21
# speedup vs baseline: 1.8206x; 1.5100x over previous
"""Trainium2 Bass kernel for CoRA/AdaLoRA embedding lookup.

Computes: out = (E + scaling * lora_B @ (lora_A * mask))[x]  for
  E [500000, 128] f32, lora_B [500000, 8] f32, lora_A [8, 128] f32,
  rank_pattern [8] f32, x [4096, 200] int.

Strategy: pure data-parallel over the batch across 8 NeuronCores with the
table replicated.  Per core, tokens are bucketed by vocab bank (16 banks of
31250 rows, so in-bank indices fit int16) on the host.  Rows of a fused
table [E | lora_B | pad] (768 B, dma_gather needs elem%256B==0) are gathered
with gpsimd.dma_gather in chunks of 1024 indices (ucode descriptor-ring
limit) with -1 tail padding (skipped by HW).  The rank-8 LoRA delta is
computed on-chip (PE transpose + block-diagonal K=64 matmul) and added in
place to the gathered embedding columns, which are DMAd out per bank.  The
host un-permutes the sorted output.  Per-core HBM traffic ~140 MB.
"""

import numpy as np

V = 500000
D = 128
R = 8
EROW = 192             # fused row: 128 emb + 8 lora_B + 56 pad = 768 B
SCALING = 2.0          # LORA_ALPHA / R = 16 / 8
THRESH = 0.1
B, L = 4096, 200
NCORES = 8
P = 128
TPC = B * L // NCORES  # 102400 tokens per core

NBANK = 16
BW = V // NBANK        # 31250 (< 2^15, in-bank index fits int16)
NSUB = 7               # chunks (= compute subtiles) per bank
G = 8                  # dst columns (of 128 slots) per chunk
CHUNK = G * P          # 1024 idxs per dma_gather (HW ring limit)
CAP = NSUB * CHUNK     # 7168 slots per bank (static capacity)
CCOL = CAP // P        # 56 dst columns per bank
ICOL = CAP // 16       # 448 idx columns per bank
NCOL = NBANK * CCOL    # 896 total out columns


def build_nc(nbank=NBANK, bw=BW, nsub=NSUB):
    from concourse import bass, bacc, mybir
    from concourse.library_config import mlp
    from contextlib import ExitStack

    f32 = mybir.dt.float32
    bf16 = mybir.dt.bfloat16
    i16 = mybir.dt.int16
    cap = nsub * CHUNK
    ccol = cap // P
    icol = cap // 16
    ichk = CHUNK // 16  # 64 idx columns per chunk
    v = nbank * bw
    nsubt = nbank * nsub  # total subtiles == total gathers

    nc = bacc.Bacc(num_swdge_queues=3)
    tab = nc.declare_dram_parameter("tab", [v, EROW], f32, False)
    idx = nc.declare_dram_parameter("idx", [P, nbank * icol], i16, False)
    cnts = nc.declare_dram_parameter("cnts", [1, nbank * nsub], mybir.dt.int32, False)
    aeffb = nc.declare_dram_parameter("aeffb", [G * R, G * D], bf16, False)
    ident = nc.declare_dram_parameter("ident", [P, P], f32, False)
    out = nc.declare_dram_parameter("out", [P, nbank * ccol, D], f32, True)

    with ExitStack() as st:
        block = st.enter_context(nc.Block())
        idx_sb = st.enter_context(nc.sbuf_tensor("idx_sb", [P, nbank * icol], i16))
        cnts_sb = st.enter_context(
            nc.sbuf_tensor("cnts_sb", [1, nbank * nsub], mybir.dt.int32)
        )
        aug = [
            st.enter_context(nc.sbuf_tensor(f"aug{i}", [P, ccol, EROW], f32))
            for i in range(3)
        ]
        lb_cont = [
            st.enter_context(nc.sbuf_tensor(f"lbc{i}", [P, G * R], f32))
            for i in range(2)
        ]
        lbT = [
            st.enter_context(nc.sbuf_tensor(f"lbT{i}", [G * R, P], bf16))
            for i in range(2)
        ]
        ident_sb = st.enter_context(nc.sbuf_tensor("ident_sb", [P, P], f32))
        aeff_sb = st.enter_context(nc.sbuf_tensor("aeff_sb", [G * R, G * D], bf16))
        pt_full = [
            st.enter_context(nc.psum_tensor(f"pt{i}", [G * R, 512], f32))
            for i in range(2)
        ]
        pm = [
            [
                st.enter_context(nc.psum_tensor(f"pm{i}_{q}", [P, 512], f32))
                for q in range(2)
            ]
            for i in range(2)
        ]
        io_sem = st.enter_context(nc.semaphore("io_sem"))
        ix_sem = st.enter_context(nc.semaphore("ix_sem"))
        z_sem = st.enter_context(nc.semaphore("z_sem"))
        g_sems = [st.enter_context(nc.semaphore(f"g_sem{i}")) for i in range(3)]
        o_sem = st.enter_context(nc.semaphore("o_sem"))
        d1_sem = st.enter_context(nc.semaphore("d1_sem"))
        d2_sem = st.enter_context(nc.semaphore("d2_sem"))
        d3_sem = st.enter_context(nc.semaphore("d3_sem"))
        pe_sem = st.enter_context(nc.semaphore("pe_sem"))

        @block.gpsimd
        def _(gp: "bass.BassGpSimd"):
            gp.load_library(mlp)
            gp.wait_ge(ix_sem, 32)  # idx + counts loaded
            gp.wait_ge(z_sem, 3)    # aug tiles zeroed
            with gp.register("cnt") as cnt_reg:
                for b in range(nbank):
                    pe_ = b % 3
                    if b >= 3:
                        gp.wait_ge(o_sem, 16 * (b - 2))  # out DMA of bank b-3 done
                    for s in range(nsub):
                        k = b * nsub + s
                        gp.reg_load(cnt_reg, cnts_sb[0:1, k : k + 1])
                        cnt = gp.snap(cnt_reg)
                        gp.dma_gather(
                            aug[pe_][:, s * G : (s + 1) * G, :],
                            tab[b * bw : (b + 1) * bw, :],
                            idx_sb[:, b * icol + s * ichk : b * icol + (s + 1) * ichk],
                            CHUNK,
                            cnt,
                            EROW,
                            queue_num=pe_,
                        ).then_inc(g_sems[pe_], 16)

        @block.vector
        def _(ve: "bass.BassVectorEngine"):
            for i in range(3):
                ve.memset(aug[i][:, :, :], 0.0).then_inc(z_sem, 1)
            # prologue: lb_cont for subtile 0 (whole bank 0 gathered)
            ve.wait_ge(g_sems[0], 16 * nsub)
            ve.tensor_copy(
                out=lb_cont[0][:, :], in_=aug[0][:, 0:G, D : D + R]
            ).then_inc(d1_sem, 1)
            for n in range(nsubt):
                b, s = divmod(n, nsub)
                pe_ = b % 3
                # lbT copy (needs PE transpose n)
                ve.wait_ge(pe_sem, 2 * n + 1)
                ve.tensor_copy(
                    out=lbT[n % 2][:, :], in_=pt_full[n % 2][:, 0:P]
                ).then_inc(d2_sem, 1)
                # software-pipelined lb_cont for subtile n+1
                if n + 1 < nsubt:
                    b2, s2 = divmod(n + 1, nsub)
                    if s2 == 0:
                        ve.wait_ge(g_sems[b2 % 3], 16 * nsub * (b2 // 3 + 1))
                    ve.tensor_copy(
                        out=lb_cont[(n + 1) % 2][:, :],
                        in_=aug[b2 % 3][:, s2 * G : (s2 + 1) * G, D : D + R],
                    ).then_inc(d1_sem, 1)
                # adds (need PE matmuls n); in-place into the emb columns
                ve.wait_ge(pe_sem, 2 * n + 2)
                half = G // 2
                ve.tensor_add(
                    out=aug[pe_][:, s * G : s * G + half, 0:D],
                    in0=aug[pe_][:, s * G : s * G + half, 0:D],
                    in1=pm[n % 2][0][:, :],
                )
                ve.tensor_add(
                    out=aug[pe_][:, s * G + half : (s + 1) * G, 0:D],
                    in0=aug[pe_][:, s * G + half : (s + 1) * G, 0:D],
                    in1=pm[n % 2][1][:, :],
                ).then_inc(d3_sem, 1)

        @block.tensor
        def _(te: "bass.BassTensorEngine"):
            te.wait_ge(io_sem, 32)  # ident + aeff loaded
            for n in range(nsubt):
                te.wait_ge(d1_sem, n + 1)
                if n >= 2:
                    te.wait_ge(d2_sem, n - 1)  # WAR pt[n%2]
                te.transpose(
                    out=pt_full[n % 2][:, 0:P],
                    in_=lb_cont[n % 2][:, :],
                    identity=ident_sb[:, :],
                ).then_inc(pe_sem, 1)
                te.wait_ge(d2_sem, n + 1)      # lbT ready
                if n >= 2:
                    te.wait_ge(d3_sem, n - 1)  # WAR pm[n%2]
                te.matmul(
                    out=pm[n % 2][0][:, :],
                    lhsT=lbT[n % 2][:, :],
                    rhs=aeff_sb[:, 0:512],
                    start=True,
                    stop=True,
                )
                te.matmul(
                    out=pm[n % 2][1][:, :],
                    lhsT=lbT[n % 2][:, :],
                    rhs=aeff_sb[:, 512:1024],
                    start=True,
                    stop=True,
                ).then_inc(pe_sem, 1)

        @block.sync
        def _(sy: "bass.BassEngine"):
            sy.dma_start(out=idx_sb[:, :], in_=idx[:, :]).then_inc(ix_sem, 16)
            sy.dma_start(out=cnts_sb[:, :], in_=cnts[:, :]).then_inc(ix_sem, 16)
            sy.dma_start(out=ident_sb[:, :], in_=ident[:, :]).then_inc(io_sem, 16)
            sy.dma_start(out=aeff_sb[:, :], in_=aeffb[:, :]).then_inc(io_sem, 16)
            for b in range(nbank):
                sy.wait_ge(d3_sem, nsub * (b + 1))
                sy.dma_start(
                    out=out[:, b * ccol : (b + 1) * ccol, :],
                    in_=aug[b % 3][:, :, 0:D],
                ).then_inc(o_sem, 16)
            sy.wait_ge(o_sem, 16 * nbank)

    nc.compile()
    return nc


_NC_CACHE = {}


def _get_nc():
    if "nc" not in _NC_CACHE:
        _NC_CACHE["nc"] = build_nc()
    return _NC_CACHE["nc"]


def _wrap16(lst):
    """Token t -> (t % 16, t // 16), tiled 8x across 128 partitions."""
    blk = lst.reshape(-1, 16).T  # [16, n/16]
    return np.tile(blk, (8, 1))


def prepare_in_maps(x, embedding_weight, lora_A, lora_B, rank_pattern):
    x = np.asarray(x)
    E = np.asarray(embedding_weight, dtype=np.float32)
    A = np.asarray(lora_A, dtype=np.float32)
    LB = np.asarray(lora_B, dtype=np.float32)
    rp = np.asarray(rank_pattern, dtype=np.float32)

    import ml_dtypes

    a_scaled = A * (rp > THRESH).astype(np.float32)[:, None] * np.float32(SCALING)
    aeffb = np.zeros((G * R, G * D), dtype=ml_dtypes.bfloat16)
    for gg in range(G):
        aeffb[gg * R : (gg + 1) * R, gg * D : (gg + 1) * D] = a_scaled
    tab = np.zeros((V, EROW), dtype=np.float32)
    tab[:, :D] = E
    tab[:, D : D + R] = LB
    ident = np.eye(P, dtype=np.float32)

    xi = x.reshape(-1).astype(np.int64)
    in_maps = []
    host_info = []
    for c in range(NCORES):
        xc = xi[c * TPC : (c + 1) * TPC]
        bank = xc // BW
        within = (xc - bank * BW).astype(np.int16)
        order = np.argsort(bank, kind="stable")
        counts = np.bincount(bank, minlength=NBANK).astype(np.int64)
        overflow = {}
        idx16 = np.full((P, NBANK * ICOL), -1, dtype=np.int16)
        takes = np.zeros(NBANK * NSUB, dtype=np.int32)
        start = 0
        for b in range(NBANK):
            nb = int(counts[b])
            take = min(nb, CAP)
            lst = np.full(CAP, -1, dtype=np.int16)
            lst[:take] = within[order[start : start + take]]
            if nb > CAP:  # pathological: handle the excess on the host
                overflow[b] = order[start + take : start + nb]
            # per-chunk valid counts (chunks are filled front to back)
            for s in range(NSUB):
                t = min(max(take - s * CHUNK, 0), CHUNK)
                if t == 0:  # ucode needs >=1 valid index; slot is discarded
                    lst[s * CHUNK] = 0
                    t = 1
                takes[b * NSUB + s] = t
            idx16[:, b * ICOL : (b + 1) * ICOL] = _wrap16(lst)
            start += nb
        in_maps.append(
            {
                "tab": tab,
                "idx": idx16,
                "cnts": takes.reshape(1, NBANK * NSUB),
                "aeffb": aeffb,
                "ident": ident,
            }
        )
        host_info.append((order, counts, overflow))
    return in_maps, host_info, (E, LB, a_scaled)


def collect(results, host_info, tabs, x):
    """Un-sort the banked output; host-patches (never-in-practice) bank overflow."""
    E, LB, a_scaled = tabs
    xi = np.asarray(x).reshape(-1).astype(np.int64)
    cores = []
    for c in range(NCORES):
        order, counts, overflow = host_info[c]
        oc = np.asarray(results[c]["out"])
        flat = oc.transpose(1, 0, 2).reshape(NCOL * P, D)
        core_out = np.empty((TPC, D), dtype=np.float32)
        src_slots = np.concatenate(
            [np.arange(min(int(counts[b]), CAP)) + b * CAP for b in range(NBANK)]
        )
        starts = np.concatenate([[0], np.cumsum(counts)]).astype(np.int64)
        dst_tok = np.concatenate(
            [order[starts[b] : starts[b] + min(int(counts[b]), CAP)] for b in range(NBANK)]
        )
        core_out[dst_tok] = flat[src_slots]
        for b, toks in overflow.items():
            ids = xi[c * TPC + toks]
            core_out[toks] = E[ids] + LB[ids] @ a_scaled
        cores.append(core_out)
    return np.concatenate(cores, axis=0).reshape(B, L, D)


def kernel(x, embedding_weight, lora_A, lora_B, rank_pattern):
    from concourse.bass_utils import run_bass_kernel_spmd

    x = np.asarray(x)
    in_maps, host_info, tabs = prepare_in_maps(
        x, embedding_weight, lora_A, lora_B, rank_pattern
    )
    nc = _get_nc()
    res = run_bass_kernel_spmd(nc, in_maps, list(range(NCORES))).results
    return collect(res, host_info, tabs, x)


# revision 22
# speedup vs baseline: 1.9023x; 1.0449x over previous
"""Trainium2 Bass kernel for CoRA/AdaLoRA embedding lookup.

Computes: out = (E + scaling * lora_B @ (lora_A * mask))[x]  for
  E [500000, 128] f32, lora_B [500000, 8] f32, lora_A [8, 128] f32,
  rank_pattern [8] f32, x [4096, 200] int.

Strategy: pure data-parallel over the batch across 8 NeuronCores with the
table replicated.  Per core, tokens are bucketed by vocab bank (16 banks of
31250 rows, so in-bank indices fit int16) on the host.  Rows of a fused
table [E | lora_B | pad] (768 B, dma_gather needs elem%256B==0) are gathered
with gpsimd.dma_gather in chunks of 1024 indices (ucode descriptor-ring
limit) with -1 tail padding (skipped by HW).  The rank-8 LoRA delta is
computed on-chip (PE transpose + block-diagonal K=64 matmul) and added in
place to the gathered embedding columns, which are DMAd out per bank.  The
host un-permutes the sorted output.  Per-core HBM traffic ~140 MB.
"""

import numpy as np

V = 500000
D = 128
R = 8
EROW = 192             # fused row: 128 emb + 8 lora_B + 56 pad = 768 B
SCALING = 2.0          # LORA_ALPHA / R = 16 / 8
THRESH = 0.1
B, L = 4096, 200
NCORES = 8
P = 128
TPC = B * L // NCORES  # 102400 tokens per core

NBANK = 16
BW = V // NBANK        # 31250 (< 2^15, in-bank index fits int16)
NSUB = 7               # chunks (= compute subtiles) per bank
G = 8                  # dst columns (of 128 slots) per chunk
CHUNK = G * P          # 1024 idxs per dma_gather (HW ring limit)
CAP = NSUB * CHUNK     # 7168 slots per bank (static capacity)
CCOL = CAP // P        # 56 dst columns per bank
ICOL = CAP // 16       # 448 idx columns per bank
NCOL = NBANK * CCOL    # 896 total out columns


def build_nc(nbank=NBANK, bw=BW, nsub=NSUB):
    from concourse import bass, bacc, mybir
    from concourse.library_config import mlp
    from contextlib import ExitStack

    f32 = mybir.dt.float32
    bf16 = mybir.dt.bfloat16
    i16 = mybir.dt.int16
    cap = nsub * CHUNK
    ccol = cap // P
    icol = cap // 16
    ichk = CHUNK // 16  # 64 idx columns per chunk
    v = nbank * bw
    nsubt = nbank * nsub  # total subtiles == total gathers

    nc = bacc.Bacc(num_swdge_queues=3)
    tab = nc.declare_dram_parameter("tab", [v, EROW], f32, False)
    idx = nc.declare_dram_parameter("idx", [P, nbank * icol], i16, False)
    cnts = nc.declare_dram_parameter("cnts", [1, nbank * nsub], mybir.dt.int32, False)
    aeffb = nc.declare_dram_parameter("aeffb", [G * R, G * D], bf16, False)
    ident = nc.declare_dram_parameter("ident", [P, P], f32, False)
    out = nc.declare_dram_parameter("out", [P, nbank * ccol, D], f32, True)

    with ExitStack() as st:
        block = st.enter_context(nc.Block())
        idx_sb = st.enter_context(nc.sbuf_tensor("idx_sb", [P, nbank * icol], i16))
        cnts_sb = st.enter_context(
            nc.sbuf_tensor("cnts_sb", [1, nbank * nsub], mybir.dt.int32)
        )
        aug = [
            st.enter_context(nc.sbuf_tensor(f"aug{i}", [P, ccol, EROW], f32))
            for i in range(3)
        ]
        lb_cont = [
            st.enter_context(nc.sbuf_tensor(f"lbc{i}", [P, G * R], f32))
            for i in range(2)
        ]
        lbT = [
            st.enter_context(nc.sbuf_tensor(f"lbT{i}", [G * R, P], bf16))
            for i in range(2)
        ]
        ident_sb = st.enter_context(nc.sbuf_tensor("ident_sb", [P, P], f32))
        aeff_sb = st.enter_context(nc.sbuf_tensor("aeff_sb", [G * R, G * D], bf16))
        pt_full = [
            st.enter_context(nc.psum_tensor(f"pt{i}", [G * R, 512], f32))
            for i in range(2)
        ]
        pm = [
            [
                st.enter_context(nc.psum_tensor(f"pm{i}_{q}", [P, 512], f32))
                for q in range(2)
            ]
            for i in range(2)
        ]
        io_sem = st.enter_context(nc.semaphore("io_sem"))
        ix_sem = st.enter_context(nc.semaphore("ix_sem"))
        z_sem = st.enter_context(nc.semaphore("z_sem"))
        g_sems = [st.enter_context(nc.semaphore(f"g_sem{i}")) for i in range(3)]
        o_sem = st.enter_context(nc.semaphore("o_sem"))
        d1_sem = st.enter_context(nc.semaphore("d1_sem"))
        d2_sem = st.enter_context(nc.semaphore("d2_sem"))
        d3_sem = st.enter_context(nc.semaphore("d3_sem"))
        pe_sem = st.enter_context(nc.semaphore("pe_sem"))

        @block.gpsimd
        def _(gp: "bass.BassGpSimd"):
            gp.load_library(mlp)
            gp.wait_ge(ix_sem, 32)  # idx + counts loaded
            with gp.register("cnt") as cnt_reg:
                for b in range(nbank):
                    pe_ = b % 3
                    if b < 3:
                        gp.wait_ge(z_sem, pe_ + 1)  # aug[pe_] zeroed
                    else:
                        gp.wait_ge(o_sem, 16 * (b - 2))  # out DMA of bank b-3 done
                    for s in range(nsub):
                        k = b * nsub + s
                        gp.reg_load(cnt_reg, cnts_sb[0:1, k : k + 1])
                        cnt = gp.snap(cnt_reg)
                        gp.dma_gather(
                            aug[pe_][:, s * G : (s + 1) * G, :],
                            tab[b * bw : (b + 1) * bw, :],
                            idx_sb[:, b * icol + s * ichk : b * icol + (s + 1) * ichk],
                            CHUNK,
                            cnt,
                            EROW,
                            queue_num=pe_,
                        ).then_inc(g_sems[pe_], 16)

        @block.vector
        def _(ve: "bass.BassVectorEngine"):
            for i in range(3):
                ve.memset(aug[i][:, :, :], 0.0).then_inc(z_sem, 1)
            # prologue: lb_cont for subtile 0 (whole bank 0 gathered)
            ve.wait_ge(g_sems[0], 16 * nsub)
            ve.tensor_copy(
                out=lb_cont[0][:, :], in_=aug[0][:, 0:G, D : D + R]
            ).then_inc(d1_sem, 1)
            for n in range(nsubt):
                b, s = divmod(n, nsub)
                pe_ = b % 3
                # lbT copy (needs PE transpose n)
                ve.wait_ge(pe_sem, 2 * n + 1)
                ve.tensor_copy(
                    out=lbT[n % 2][:, :], in_=pt_full[n % 2][:, 0:P]
                ).then_inc(d2_sem, 1)
                # software-pipelined lb_cont for subtile n+1
                if n + 1 < nsubt:
                    b2, s2 = divmod(n + 1, nsub)
                    if s2 == 0:
                        ve.wait_ge(g_sems[b2 % 3], 16 * nsub * (b2 // 3 + 1))
                    ve.tensor_copy(
                        out=lb_cont[(n + 1) % 2][:, :],
                        in_=aug[b2 % 3][:, s2 * G : (s2 + 1) * G, D : D + R],
                    ).then_inc(d1_sem, 1)
                # adds (need PE matmuls n); in-place into the emb columns
                ve.wait_ge(pe_sem, 2 * n + 2)
                half = G // 2
                ve.tensor_add(
                    out=aug[pe_][:, s * G : s * G + half, 0:D],
                    in0=aug[pe_][:, s * G : s * G + half, 0:D],
                    in1=pm[n % 2][0][:, :],
                )
                ve.tensor_add(
                    out=aug[pe_][:, s * G + half : (s + 1) * G, 0:D],
                    in0=aug[pe_][:, s * G + half : (s + 1) * G, 0:D],
                    in1=pm[n % 2][1][:, :],
                ).then_inc(d3_sem, 1)

        @block.tensor
        def _(te: "bass.BassTensorEngine"):
            te.wait_ge(io_sem, 32)  # ident + aeff loaded
            for n in range(nsubt):
                te.wait_ge(d1_sem, n + 1)
                if n >= 2:
                    te.wait_ge(d2_sem, n - 1)  # WAR pt[n%2]
                te.transpose(
                    out=pt_full[n % 2][:, 0:P],
                    in_=lb_cont[n % 2][:, :],
                    identity=ident_sb[:, :],
                ).then_inc(pe_sem, 1)
                te.wait_ge(d2_sem, n + 1)      # lbT ready
                if n >= 2:
                    te.wait_ge(d3_sem, n - 1)  # WAR pm[n%2]
                te.matmul(
                    out=pm[n % 2][0][:, :],
                    lhsT=lbT[n % 2][:, :],
                    rhs=aeff_sb[:, 0:512],
                    start=True,
                    stop=True,
                )
                te.matmul(
                    out=pm[n % 2][1][:, :],
                    lhsT=lbT[n % 2][:, :],
                    rhs=aeff_sb[:, 512:1024],
                    start=True,
                    stop=True,
                ).then_inc(pe_sem, 1)

        @block.sync
        def _(sy: "bass.BassEngine"):
            sy.dma_start(out=idx_sb[:, :], in_=idx[:, :]).then_inc(ix_sem, 16)
            sy.dma_start(out=cnts_sb[:, :], in_=cnts[:, :]).then_inc(ix_sem, 16)
            sy.dma_start(out=ident_sb[:, :], in_=ident[:, :]).then_inc(io_sem, 16)
            sy.dma_start(out=aeff_sb[:, :], in_=aeffb[:, :]).then_inc(io_sem, 16)
            for b in range(nbank):
                sy.wait_ge(d3_sem, nsub * (b + 1))
                sy.dma_start(
                    out=out[:, b * ccol : (b + 1) * ccol, :],
                    in_=aug[b % 3][:, :, 0:D],
                ).then_inc(o_sem, 16)
            sy.wait_ge(o_sem, 16 * nbank)

    nc.compile()
    return nc


_NC_CACHE = {}


def _get_nc():
    if "nc" not in _NC_CACHE:
        _NC_CACHE["nc"] = build_nc()
    return _NC_CACHE["nc"]


def _wrap16(lst):
    """Token t -> (t % 16, t // 16), tiled 8x across 128 partitions."""
    blk = lst.reshape(-1, 16).T  # [16, n/16]
    return np.tile(blk, (8, 1))


def prepare_in_maps(x, embedding_weight, lora_A, lora_B, rank_pattern):
    x = np.asarray(x)
    E = np.asarray(embedding_weight, dtype=np.float32)
    A = np.asarray(lora_A, dtype=np.float32)
    LB = np.asarray(lora_B, dtype=np.float32)
    rp = np.asarray(rank_pattern, dtype=np.float32)

    import ml_dtypes

    a_scaled = A * (rp > THRESH).astype(np.float32)[:, None] * np.float32(SCALING)
    aeffb = np.zeros((G * R, G * D), dtype=ml_dtypes.bfloat16)
    for gg in range(G):
        aeffb[gg * R : (gg + 1) * R, gg * D : (gg + 1) * D] = a_scaled
    tab = np.zeros((V, EROW), dtype=np.float32)
    tab[:, :D] = E
    tab[:, D : D + R] = LB
    ident = np.eye(P, dtype=np.float32)

    xi = x.reshape(-1).astype(np.int64)
    in_maps = []
    host_info = []
    for c in range(NCORES):
        xc = xi[c * TPC : (c + 1) * TPC]
        bank = xc // BW
        within = (xc - bank * BW).astype(np.int16)
        order = np.argsort(bank, kind="stable")
        counts = np.bincount(bank, minlength=NBANK).astype(np.int64)
        overflow = {}
        idx16 = np.full((P, NBANK * ICOL), -1, dtype=np.int16)
        takes = np.zeros(NBANK * NSUB, dtype=np.int32)
        start = 0
        for b in range(NBANK):
            nb = int(counts[b])
            take = min(nb, CAP)
            lst = np.full(CAP, -1, dtype=np.int16)
            lst[:take] = within[order[start : start + take]]
            if nb > CAP:  # pathological: handle the excess on the host
                overflow[b] = order[start + take : start + nb]
            # per-chunk valid counts (chunks are filled front to back)
            for s in range(NSUB):
                t = min(max(take - s * CHUNK, 0), CHUNK)
                if t == 0:  # ucode needs >=1 valid index; slot is discarded
                    lst[s * CHUNK] = 0
                    t = 1
                takes[b * NSUB + s] = t
            idx16[:, b * ICOL : (b + 1) * ICOL] = _wrap16(lst)
            start += nb
        in_maps.append(
            {
                "tab": tab,
                "idx": idx16,
                "cnts": takes.reshape(1, NBANK * NSUB),
                "aeffb": aeffb,
                "ident": ident,
            }
        )
        host_info.append((order, counts, overflow))
    return in_maps, host_info, (E, LB, a_scaled)


def collect(results, host_info, tabs, x):
    """Un-sort the banked output; host-patches (never-in-practice) bank overflow."""
    E, LB, a_scaled = tabs
    xi = np.asarray(x).reshape(-1).astype(np.int64)
    cores = []
    for c in range(NCORES):
        order, counts, overflow = host_info[c]
        oc = np.asarray(results[c]["out"])
        flat = oc.transpose(1, 0, 2).reshape(NCOL * P, D)
        core_out = np.empty((TPC, D), dtype=np.float32)
        src_slots = np.concatenate(
            [np.arange(min(int(counts[b]), CAP)) + b * CAP for b in range(NBANK)]
        )
        starts = np.concatenate([[0], np.cumsum(counts)]).astype(np.int64)
        dst_tok = np.concatenate(
            [order[starts[b] : starts[b] + min(int(counts[b]), CAP)] for b in range(NBANK)]
        )
        core_out[dst_tok] = flat[src_slots]
        for b, toks in overflow.items():
            ids = xi[c * TPC + toks]
            core_out[toks] = E[ids] + LB[ids] @ a_scaled
        cores.append(core_out)
    return np.concatenate(cores, axis=0).reshape(B, L, D)


def kernel(x, embedding_weight, lora_A, lora_B, rank_pattern):
    from concourse.bass_utils import run_bass_kernel_spmd

    x = np.asarray(x)
    in_maps, host_info, tabs = prepare_in_maps(
        x, embedding_weight, lora_A, lora_B, rank_pattern
    )
    nc = _get_nc()
    res = run_bass_kernel_spmd(nc, in_maps, list(range(NCORES))).results
    return collect(res, host_info, tabs, x)
